# revision 1
# baseline (speedup 1.0000x reference)
"""Trainium2 Bass kernel for nn_BinaryDiceLoss_blobPunish (B=16, H=W=512).

Reference semantics:
    thr = predict.max()/2;  mask = predict > thr
    labels = 200 iters of masked 3x3 max-pool label propagation
    n_unique = #distinct label values
    penalty = clip: n_unique/B, <1 -> B, capped at B
    dice_i = 1 - (sum(p_i t_i)+1)/(sum(p_i^2)+sum(t_i^2)+1)
    out = mean(dice_i) * penalty

Distribution: 2 images per core on 8 NeuronCores, ONE SPMD launch, no
cross-core collectives.  Each core computes per-image dice partial sums,
its local max M_c (partition_all_reduce), and an isolated-mask-pixel
count at its LOCAL threshold T_c = M_c/2, plus a "danger" count of
pixels in (T_c, M_c*0.50005].

Host-side certificate (sound): the true threshold T = M/2 satisfies
T >= T_c, and core c's true mask differs from its local mask only at
pixels in (T_c, T] -- counted by danger_c when T <= M_c*0.50004
(host-checked).  Every isolated mask pixel keeps its own unique label
under max-pool propagation, and each mask-flipped pixel changes at most
9 pixels' isolated status, so
    n_unique >= sum_c iso_c - 9*sum_c danger_c - 1.
If that bound is >= 256, penalty == 16 exactly; otherwise an exact
numpy fallback recomputes the penalty (never hit for this generator).

Isolated pixels on device:  m = mask (bf16, zero-padded borders),
H1[c] = m[c-1]+m[c+1] (DVE), then PE band matmuls build
S3x3 - 2m = T3 @ H1 + (T3 - 2I) @ m (+ U/L cross-chunk row fixes);
a pixel is isolated iff that equals -1 (DVE is_equal, ACT accumulate).
Counts of m_lo/m_hi come from ones-column matmuls on the PE.

Raw-bacc implementation (no TileContext): hand-placed semaphores, no
Tile preamble/tail barriers, no per-op event-semaphore overhead.
GPSIMD does only partition_all_reduce + border memsets (its tensor ops
starve DVE via the shared SBUF port mux).

Engine programs (all in-order per engine, <=1 sem wait per instruction):
  SP  : tri dma, 8 p-chunk dmas, 8 t-chunk dmas, 4 output dmas
  DVE : 8 chunk maxes, local-max reduce, 4 masks, 4 H1, 4 m_hi,
        4 iso is_equal, 2 z = p*t per image
  GP  : 4 mask-border memsets, partition_all_reduce(max)
  ACT : thresholds, per-image Square/Copy accumulations, iso Copy-accums,
        PSUM count-row copies
  PE  : 16 ones-count matmuls + 40 band matmuls (3 rotating PSUM tiles)
"""

from contextlib import ExitStack

import numpy as np

B = 16
H = 512
W = 512
N_CORES = 8
IPC = B // N_CORES  # images per core
RPC = IPC * H  # rows per core
NCHUNK = H // 128  # 128-row chunks per image
EPS_SCALE = 0.50005  # device danger-window threshold scale
EPS_CHECK = 0.50004  # stricter host-side coverage check


def _install_ntff_hook():
    """Make trace=True work under axon: the stub antenv package lacks
    axon_hooks, so boot() silently skipped NTFF hook registration."""
    import sys
    import types

    if "antenv.axon_hooks" in sys.modules:
        return
    try:
        import antenv

        mod = types.ModuleType("antenv.axon_hooks")
        mod._hook = None
        mod.set_axon_ntff_profile_hook = lambda h: setattr(mod, "_hook", h)
        mod.get_axon_ntff_profile_hook = lambda: mod._hook
        sys.modules["antenv.axon_hooks"] = mod
        antenv.axon_hooks = mod
        from trn_agent_boot.trn_boot import _ntff_profile_via_ctypes

        hook = _ntff_profile_via_ctypes("/opt/axon/libaxon_pjrt.so")
        if hook is not None:
            mod.set_axon_ntff_profile_hook(hook)
    except Exception:
        pass


def _tri_matrices():
    import ml_dtypes

    tri = np.zeros((128, 5 * 128), np.float32)
    idx = np.arange(128)
    T3 = tri[:, 0:128]
    T3[idx, idx] = 1.0
    T3[idx[:-1], idx[:-1] + 1] = 1.0
    T3[idx[:-1] + 1, idx[:-1]] = 1.0
    C = tri[:, 128:256]
    C[:] = T3
    C[idx, idx] = -1.0
    tri[127, 256 + 0] = 1.0  # U: lhsT[127,0]
    tri[0, 384 + 127] = 1.0  # L: lhsT[0,127]
    tri[:, 512] = 1.0  # ones column for count matmuls
    return tri.astype(ml_dtypes.bfloat16)


def _penalty_fallback(predict):
    """Exact numpy replica of the reference penalty path (rarely used)."""
    p = np.asarray(predict, np.float32).reshape(B, H, W)
    thr = np.float32(p.max()) / np.float32(2.0)
    mask = p > thr
    init = np.arange(B * H * W, dtype=np.float32).reshape(B, H, W)
    lab = np.where(mask, init, np.float32(0.0))
    pad = np.empty((B, H + 2, W + 2), np.float32)
    for _ in range(200):
        pad.fill(-np.inf)
        pad[:, 1:-1, 1:-1] = lab
        mx = pad[:, 0:-2, 0:-2]
        for dr in range(3):
            for dc in range(3):
                if dr == 0 and dc == 0:
                    continue
                mx = np.maximum(mx, pad[:, dr : dr + H, dc : dc + W])
        new = np.where(mask, mx, np.float32(0.0))
        if np.array_equal(new, lab):
            lab = new
            break
        lab = new
    n_unique = np.unique(lab).size
    penalty = np.float32(n_unique) / np.float32(B)
    if penalty < 1.0:
        penalty = np.float32(B)
    return float(min(penalty, np.float32(B)))

_cache: dict = {}
LAST_PERF: dict = {}


def _build():
    import concourse.bacc as bacc
    from concourse import bass_isa, mybir

    f32 = mybir.dt.float32
    bf16 = mybir.dt.bfloat16
    A = mybir.AluOpType
    AF = mybir.ActivationFunctionType
    X = mybir.AxisListType.X

    nc = bacc.Bacc("TRN2", target_bir_lowering=False, debug=False, num_devices=N_CORES)
    p = nc.dram_tensor("p", [RPC, W], f32, kind="ExternalInput").ap()
    t = nc.dram_tensor("t", [RPC, W], f32, kind="ExternalInput").ap()
    tri = nc.dram_tensor("tri", [128, 5 * 128], bf16, kind="ExternalInput").ap()
    acc_d = nc.dram_tensor("acc", [128, 8], f32, kind="ExternalOutput").ap()
    iso_d = nc.dram_tensor("iso", [128, NCHUNK], f32, kind="ExternalOutput").ap()
    sums_d = nc.dram_tensor("sums", [2, W], f32, kind="ExternalOutput").ap()

    p_v = p.rearrange("(n q) m -> n q m", q=128)
    t_v = t.rearrange("(n q) m -> n q m", q=128)

    NJ = IPC * NCHUNK  # 8 chunks

    with ExitStack() as ctx:
        _n = [0]

        def sb(shape, dt, name=None):
            _n[0] += 1
            return ctx.enter_context(
                nc.sbuf_tensor(name or f"sb{_n[0]}", shape, dt)
            )

        def ps(shape, name=None):
            _n[0] += 1
            return ctx.enter_context(
                nc.psum_tensor(name or f"ps{_n[0]}", shape, f32)
            )

        def sem(name):
            return ctx.enter_context(nc.semaphore(name))

        tri_t = sb([128, 5 * 128], bf16)
        p_res = sb([128, NJ, W], f32)
        t_res = sb([128, NJ, W], f32)
        z_res = sb([128, NJ, W], f32)
        mp_all = sb([128, NCHUNK, IPC, W + 2], bf16)
        mp = [mp_all[:, k, :, :] for k in range(NCHUNK)]
        h1_all = sb([128, NCHUNK, IPC, W], bf16)
        h1 = [h1_all[:, k, :, :] for k in range(NCHUNK)]
        mhi_all = sb([128, NCHUNK, IPC, W], bf16)
        mhi = [mhi_all[:, k, :, :] for k in range(NCHUNK)]
        ind = sb([128, NCHUNK, IPC * W], bf16)
        acc = sb([128, 8], f32)
        iso_acc = sb([128, NCHUNK], f32)
        mx8 = sb([128, NJ], f32)
        mxc = sb([128, 1], f32)
        mx_all = sb([128, 1], f32)
        thr_t = sb([128, 1], f32)
        thr2_t = sb([128, 1], f32)
        mlo_sb = sb([1, W], f32)
        mhi_sb = sb([1, W], f32)
        sq_scr = sb([128, NCHUNK * W], bf16)

        psA = [ps([128, IPC * W]) for _ in range(3)]  # 6 banks
        mlo_ps = ps([1, W])
        mhi_ps = ps([1, W])

        s_p = [sem(f"s_p{j}") for j in range(NJ)]
        s_t = [sem(f"s_t{j}") for j in range(NJ)]
        s_mx8 = sem("s_mx8")
        s_zc = sem("s_zc")
        s_tri = sem("s_tri")
        s_mset = sem("s_mset")
        s_mx = sem("s_mx")
        s_allred = sem("s_allred")
        s_thr = sem("s_thr")
        s_mask = sem("s_mask")
        s_h1 = sem("s_h1")
        s_mhi = sem("s_mhi")
        s_eq = sem("s_eq")
        s_psA = sem("s_psA")
        s_cnt = sem("s_cnt")
        s_z = sem("s_z")
        s_iso = sem("s_iso")
        s_act = sem("s_act")
        s_out = sem("s_out")

        with nc.Block() as block:

            @block.sync
            def _(sync):
                sync.dma_start(tri_t[:], tri[:]).then_inc(s_tri, 16)
                for j in range(NJ):
                    sync.dma_start(p_res[:, j, :], p_v[j]).then_inc(s_p[j], 16)
                for j in range(NJ):
                    sync.dma_start(t_res[:, j, :], t_v[j]).then_inc(s_t[j], 16)
                sync.wait_ge(s_act, 1)
                sync.dma_start(acc_d[:], acc[:]).then_inc(s_out, 16)
                sync.wait_ge(s_cnt, 4)
                sync.dma_start(sums_d[0:1, :], mlo_sb[:]).then_inc(s_out, 16)
                sync.dma_start(sums_d[1:2, :], mhi_sb[:]).then_inc(s_out, 16)
                sync.wait_ge(s_iso, NCHUNK)
                sync.dma_start(iso_d[:], iso_acc[:]).then_inc(s_out, 16)

            @block.gpsimd
            def _(gpsimd):
                nc.gpsimd.memset(
                    mp_all[:, :, :, 0 : W + 2 : W + 1], 0.0
                ).then_inc(s_mset, 1)
                gpsimd.wait_ge(s_mx, 1)
                nc.gpsimd.partition_all_reduce(
                    mx_all[:], mxc[:], channels=128, reduce_op=bass_isa.ReduceOp.max
                ).then_inc(s_allred, 1)

            @block.vector
            def _(vector):
                def z_op(j):
                    vector.wait_ge(s_t[j], 16)
                    nc.vector.tensor_mul(
                        z_res[:, j, :], p_res[:, j, :], t_res[:, j, :]
                    ).then_inc(s_zc, 1)

                for j in range(NJ):
                    vector.wait_ge(s_p[j], 16)
                    nc.vector.reduce_max(
                        mx8[:, j : j + 1], p_res[:, j, :], axis=X
                    ).then_inc(s_mx8, 1)
                nc.vector.reduce_max(mxc[:], mx8[:], axis=X).then_inc(s_mx, 1)
                p4 = p_res[:, :, :].rearrange("q (i k) m -> q k i m", i=IPC)
                vector.wait_ge(s_thr, 1)
                nc.vector.tensor_scalar(
                    mp_all[:, :, :, 1 : W + 1], p4, thr_t[:], None, A.is_gt
                ).then_inc(s_mask, 1)
                for k in range(NCHUNK):
                    if k == 0:
                        vector.wait_ge(s_mset, 1)
                    nc.vector.tensor_add(
                        h1[k][:], mp[k][:, :, 0:W], mp[k][:, :, 2 : W + 2]
                    ).then_inc(s_h1, 1)
                nc.vector.tensor_scalar(
                    mhi_all[:], p4, thr2_t[:], None, A.is_gt
                ).then_inc(s_mhi, 1)

                def eq_op(k):
                    vector.wait_ge(s_psA, k + 1)
                    nc.vector.tensor_scalar(
                        ind[:, k, :], psA[k % 3][:], -1.0, None, A.is_equal
                    ).then_inc(s_eq, 1)

                for j in range(NCHUNK):
                    z_op(j)
                eq_op(0)
                z_op(4)
                z_op(5)
                eq_op(1)
                z_op(6)
                z_op(7)
                eq_op(2)
                eq_op(3)

            @block.scalar
            def _(scalar):
                for i in range(IPC):
                    blk = slice(i * NCHUNK, (i + 1) * NCHUNK)
                    scalar.wait_ge(s_mx8, NCHUNK * (i + 1))
                    nc.scalar.activation(
                        sq_scr[:],
                        p_res[:, blk, :].rearrange("q n m -> q (n m)"),
                        AF.Square,
                        accum_out=acc[:, i : i + 1],
                    )
                scalar.wait_ge(s_allred, 1)
                nc.scalar.activation(thr_t[:], mx_all[:], AF.Copy, bias=0.0, scale=0.5)
                nc.scalar.activation(
                    thr2_t[:], mx_all[:], AF.Copy, bias=0.0, scale=float(EPS_SCALE)
                ).then_inc(s_thr, 1)
                for i in range(IPC):
                    blk = slice(i * NCHUNK, (i + 1) * NCHUNK)
                    scalar.wait_ge(s_zc, NCHUNK * (i + 1))
                    nc.scalar.activation(
                        sq_scr[:],
                        t_res[:, blk, :].rearrange("q n m -> q (n m)"),
                        AF.Square,
                        accum_out=acc[:, 2 + i : 3 + i],
                    )
                for i in range(IPC):
                    blk = slice(i * NCHUNK, (i + 1) * NCHUNK)
                    scalar.wait_ge(s_zc, NCHUNK * (i + 1))
                    nc.scalar.activation(
                        sq_scr[:],
                        z_res[:, blk, :].rearrange("q n m -> q (n m)"),
                        AF.Copy,
                        bias=0.0,
                        scale=1.0,
                        accum_out=acc[:, 4 + i : 5 + i],
                    )
                nc.scalar.copy(acc[:, 6:7], mxc[:])
                nc.scalar.copy(acc[:, 7:8], mxc[:]).then_inc(s_act, 1)
                scalar.wait_ge(s_cnt, 2)
                nc.scalar.copy(mlo_sb[:], mlo_ps[:])
                nc.scalar.copy(mhi_sb[:], mhi_ps[:]).then_inc(s_cnt, 2)
                for k in range(NCHUNK):
                    scalar.wait_ge(s_eq, k + 1)
                    nc.scalar.activation(
                        sq_scr[:, 0 : IPC * W],
                        ind[:, k, :],
                        AF.Copy,
                        bias=0.0,
                        scale=1.0,
                        accum_out=iso_acc[:, k : k + 1],
                    ).then_inc(s_iso, 1)

            @block.tensor
            def _(tensor):
                ones_col = tri_t[:, 512:513]
                tensor.wait_ge(s_tri, 16)
                def count_k(k):
                    if k == 0:
                        tensor.wait_ge(s_mask, 1)
                    for i in range(IPC):
                        nc.tensor.matmul(
                            mlo_ps[:],
                            ones_col,
                            mp[k][:, i, 1 : W + 1],
                            start=(k == 0 and i == 0),
                            stop=(k == NCHUNK - 1 and i == IPC - 1),
                            skip_group_check=True,
                        )

                def mhi_counts():
                    tensor.wait_ge(s_mhi, 1)
                    for kk in range(NCHUNK):
                        for ii in range(IPC):
                            mm = nc.tensor.matmul(
                                mhi_ps[:],
                                ones_col,
                                mhi[kk][:, ii, :],
                                start=(kk == 0 and ii == 0),
                                stop=(kk == NCHUNK - 1 and ii == IPC - 1),
                                skip_group_check=True,
                            )
                            if kk == NCHUNK - 1 and ii == IPC - 1:
                                mm.then_inc(s_cnt, 2)

                for k in range(NCHUNK):
                    count_k(k)
                    if k == 2:
                        mhi_counts()
                    dst = psA[k % 3]
                    for i in range(IPC):  # dst columns [i*W:(i+1)*W]
                        mms = [
                            (tri_t[:, 0:128], h1[k][:, i, :], s_h1, k + 1),
                            (tri_t[:, 128:256], mp[k][:, i, 1 : W + 1], None, 0),
                        ]
                        if k > 0:
                            mms.append((tri_t[:, 256:384], h1[k - 1][:, i, :], None, 0))
                            mms.append(
                                (tri_t[:, 256:384], mp[k - 1][:, i, 1 : W + 1], None, 0)
                            )
                        if k < NCHUNK - 1:
                            mms.append(
                                (tri_t[:, 384:512], h1[k + 1][:, i, :], s_h1, k + 2)
                            )
                            mms.append(
                                (tri_t[:, 384:512], mp[k + 1][:, i, 1 : W + 1], None, 0)
                            )
                        if k >= 3 and i == 0:
                            # psum tile reuse: wait for iso eq of chunk k-3
                            tensor.wait_ge(s_eq, k - 2)
                        for q, (lhsT, rhs, wsem, wval) in enumerate(mms):
                            if wsem is not None:
                                tensor.wait_ge(wsem, wval)
                            mm = nc.tensor.matmul(
                                dst[:, i * W : (i + 1) * W],
                                lhsT,
                                rhs,
                                start=(q == 0),
                                stop=(q == len(mms) - 1),
                                skip_group_check=True,
                            )
                        if i == IPC - 1:
                            mm.then_inc(s_psA, 1)


        nc.compile()
    return nc


def _get_built():
    if "nc" not in _cache:
        _cache["nc"] = _build()
    return _cache["nc"]


def kernel(predict, target):
    import os

    from concourse.bass_utils import run_bass_kernel_spmd

    trace = bool(os.environ.get("BDICE_TRACE"))
    if trace:
        _install_ntff_hook()

    pred = np.ascontiguousarray(np.asarray(predict, np.float32).reshape(B * H, W))
    targ = np.ascontiguousarray(np.asarray(target, np.float32).reshape(B * H, W))
    p_sh = pred.reshape(N_CORES, RPC, W)
    t_sh = targ.reshape(N_CORES, RPC, W)

    nc = _get_built()
    core_ids = list(range(N_CORES))
    tri = _tri_matrices()
    in_maps = [{"p": p_sh[c], "t": t_sh[c], "tri": tri} for c in range(N_CORES)]
    res = run_bass_kernel_spmd(nc, in_maps, core_ids=core_ids, trace=trace)
    if trace:
        LAST_PERF.update(
            a_ns=res.exec_time_ns,
            b_ns=0,
            a_trace=(res.instructions_and_trace or (None, None))[1],
            b_trace=None,
        )

    acc = np.stack([res.results[c]["acc"] for c in range(N_CORES)])
    iso = np.stack([res.results[c]["iso"] for c in range(N_CORES)])
    sums = np.stack([res.results[c]["sums"] for c in range(N_CORES)])

    mc = acc[:, :, 6].max(axis=1)
    M = np.float32(mc.max())
    thr_true = M / np.float32(2.0)

    iso_total = float(iso.sum(dtype=np.float64))
    mlo = sums[:, 0, :].sum(axis=1, dtype=np.float64)
    mhi_c = sums[:, 1, :].sum(axis=1, dtype=np.float64)
    danger_total = float((mlo - mhi_c).sum())

    covered = all(
        thr_true <= np.float32(mc[c]) * np.float32(EPS_CHECK) for c in range(N_CORES)
    )
    if covered and iso_total - 9.0 * danger_total >= 257.0:
        penalty = 16.0
    else:
        penalty = _penalty_fallback(pred)

    acc64 = acc.astype(np.float64)
    losses = []
    for c in range(N_CORES):
        for i in range(IPC):
            p2 = acc64[c, :, i].sum()
            t2 = acc64[c, :, 2 + i].sum()
            pt = acc64[c, :, 4 + i].sum()
            losses.append(1.0 - (pt + 1.0) / (p2 + t2 + 1.0))
    mean_loss = float(np.mean(losses))
    return np.float32(mean_loss * penalty)



# revision 11
# speedup vs baseline: 1.8145x; 1.8145x over previous
"""Trainium2 Bass kernel for nn_BinaryDiceLoss_blobPunish (B=16, H=W=512).

Reference semantics:
    thr = predict.max()/2;  mask = predict > thr
    labels = 200 iters of masked 3x3 max-pool label propagation
    n_unique = #distinct label values
    penalty = clip: n_unique/B, <1 -> B, capped at B
    dice_i = 1 - (sum(p_i t_i)+1)/(sum(p_i^2)+sum(t_i^2)+1)
    out = mean(dice_i) * penalty

Distribution: 2 images per core on 8 NeuronCores, ONE SPMD launch, no
cross-core collectives.

Host computes the EXACT threshold thr = max(predict)/2 in f32 (bit-
identical to the reference) and ships it as a tiny input, so the
device mask is exact.  Every isolated mask pixel (no 8-neighbors set)
keeps its own unique label under max-pool propagation, and background
0 is present whenever an isolated pixel exists, so
    n_unique >= iso_count + 1.
The device counts isolated pixels on rows 0..126 of the first two
128-row chunks of image 0 on each core (exact on those rows; row 127
is excluded via a zeroed lane in the ones column).  Expected count
~2100 >> 255; if the count ever drops below 255 an exact numpy
fallback recomputes the penalty (never hit for this generator).

Dice sums:  ACT Square+accum per t/p pair-half gives sum(t^2)/sum(p^2)
pipelined against the staggered t/p DMA arrivals.  DVE tensor_mul
(bf16 out) + PE ones-column matmuls accumulate sum(p*t) into PSUM;
the final chunk's z is reduced directly on DVE to keep the tail short.
Isolated-pixel test: m = mask (bf16, zero-padded borders),
H1 = m_left + m_right (DVE), then PE band matmuls build
S3x3 - 2m = T3 @ H1 + (T3 - 2I) @ m (+ U cross-chunk row fix);
a pixel is isolated iff that equals -1 (DVE is_equal), counted with a
ones(0..126) column matmul into PSUM and a final DVE reduce.

Raw-bacc implementation (no TileContext): hand-placed semaphores.
All input DMAs ride ONE SP hardware queue (FIFO completion): 2-chunk
pairs (t-half then p-half) for chunks 0-5, then single chunks 6,7 so
the tail work is fine-grained.  Kernel is HBM-bound (~4.2 MB/core).

Engine programs (all in-order per engine, <=1 sem wait per instruction):
  SP  : thr, tri, t01,p01,t23,p23,t45,p45,t6,p6,t7,p7 dmas, 1 out dma
  GP  : mask border + out_sb tail-column memsets only
  DVE : mask, H1, 4 z muls, 2 iso is_equal, iso/zps/z7 reduces
  ACT : 6 pair-half + 4 chunk Square accums
  PE  : 6 cert band matmuls + 2 iso count + 5 z count matmuls
"""

from contextlib import ExitStack

import numpy as np

B = 16
H = 512
W = 512
N_CORES = 8
IPC = B // N_CORES  # images per core
RPC = IPC * H  # rows per core
NCHUNK = RPC // 128  # 8 128-row chunks per core


def _install_ntff_hook():
    """Make trace=True work under axon: the stub antenv package lacks
    axon_hooks, so boot() silently skipped NTFF hook registration."""
    import sys
    import types

    if "antenv.axon_hooks" in sys.modules:
        return
    try:
        import antenv

        mod = types.ModuleType("antenv.axon_hooks")
        mod._hook = None
        mod.set_axon_ntff_profile_hook = lambda h: setattr(mod, "_hook", h)
        mod.get_axon_ntff_profile_hook = lambda: mod._hook
        sys.modules["antenv.axon_hooks"] = mod
        antenv.axon_hooks = mod
        from trn_agent_boot.trn_boot import _ntff_profile_via_ctypes

        hook = _ntff_profile_via_ctypes("/opt/axon/libaxon_pjrt.so")
        if hook is not None:
            mod.set_axon_ntff_profile_hook(hook)
    except Exception:
        pass


def _tri_matrices():
    import ml_dtypes

    tri = np.zeros((128, 3 * 128 + 2), np.float32)
    idx = np.arange(128)
    T3 = tri[:, 0:128]
    T3[idx, idx] = 1.0
    T3[idx[:-1], idx[:-1] + 1] = 1.0
    T3[idx[:-1] + 1, idx[:-1]] = 1.0
    C = tri[:, 128:256]
    C[:] = T3
    C[idx, idx] = -1.0
    tri[127, 256 + 0] = 1.0  # U: lhsT[127,0] -> out row 0 += rhs row 127
    tri[0:127, 384] = 1.0  # ones column, row 127 zeroed (excluded rows)
    tri[:, 385] = 1.0  # full ones column for z count matmuls
    return tri.astype(ml_dtypes.bfloat16)


def _penalty_fallback(predict):
    """Exact numpy replica of the reference penalty path (rarely used)."""
    p = np.asarray(predict, np.float32).reshape(B, H, W)
    thr = np.float32(p.max()) / np.float32(2.0)
    mask = p > thr
    init = np.arange(B * H * W, dtype=np.float32).reshape(B, H, W)
    lab = np.where(mask, init, np.float32(0.0))
    pad = np.empty((B, H + 2, W + 2), np.float32)
    for _ in range(200):
        pad.fill(-np.inf)
        pad[:, 1:-1, 1:-1] = lab
        mx = pad[:, 0:-2, 0:-2]
        for dr in range(3):
            for dc in range(3):
                if dr == 0 and dc == 0:
                    continue
                mx = np.maximum(mx, pad[:, dr : dr + H, dc : dc + W])
        new = np.where(mask, mx, np.float32(0.0))
        if np.array_equal(new, lab):
            lab = new
            break
        lab = new
    n_unique = np.unique(lab).size
    penalty = np.float32(n_unique) / np.float32(B)
    if penalty < 1.0:
        penalty = np.float32(B)
    return float(min(penalty, np.float32(B)))


_cache: dict = {}
LAST_PERF: dict = {}


def _build():
    import concourse.bacc as bacc
    from concourse import mybir

    f32 = mybir.dt.float32
    bf16 = mybir.dt.bfloat16
    A = mybir.AluOpType
    AF = mybir.ActivationFunctionType
    X = mybir.AxisListType.X

    nc = bacc.Bacc("TRN2", target_bir_lowering=False, debug=False, num_devices=N_CORES)
    p = nc.dram_tensor("p", [RPC, W], f32, kind="ExternalInput").ap()
    t = nc.dram_tensor("t", [RPC, W], f32, kind="ExternalInput").ap()
    tri = nc.dram_tensor("tri", [128, 3 * 128 + 2], bf16, kind="ExternalInput").ap()
    thr = nc.dram_tensor("thr", [128, 1], f32, kind="ExternalInput").ap()
    out_d = nc.dram_tensor("out", [128, 14], f32, kind="ExternalOutput").ap()

    # partition-major views: [q=partition, n=chunk, m=col]
    p_v = p.rearrange("(n q) m -> q n m", q=128)
    t_v = t.rearrange("(n q) m -> q n m", q=128)

    with ExitStack() as ctx:
        _n = [0]

        def sb(shape, dt, name=None):
            _n[0] += 1
            return ctx.enter_context(
                nc.sbuf_tensor(name or f"sb{_n[0]}", shape, dt)
            )

        def ps(shape, name=None):
            _n[0] += 1
            return ctx.enter_context(
                nc.psum_tensor(name or f"ps{_n[0]}", shape, f32)
            )

        def sem(name):
            return ctx.enter_context(nc.semaphore(name))

        tri_t = sb([128, 3 * 128 + 2], bf16)
        thr_t = sb([128, 1], f32)
        # interleaved blocks: chunk j of p at [:, j, 0:W], t at [:, j, W:2W]
        pt = sb([128, NCHUNK, 2 * W], f32)
        mp = sb([128, 2, W + 2], bf16)  # img0 chunks 0,1 mask + borders
        h1 = sb([128, 2, W], bf16)
        ind = sb([128, 2, W], bf16)
        z_all = sb([128, 4, 2, W], bf16)  # elementwise p*t per pair
        sq_scr = sb([128, 2, W], bf16)  # ACT square scratch
        out_sb = sb([128, 14], f32)

        psA = [ps([128, W]) for _ in range(2)]
        iso_ps = ps([1, W])
        zps0 = ps([1, W])  # img0: z pairs 0,1
        zps1 = ps([1, W])  # img1: z pair 2 + chunk 6

        s_aux = sem("s_aux")
        s_td = [sem(f"s_td{j}") for j in range(4)]  # t01,t23,t45,t6
        s_pd = [sem(f"s_pd{j}") for j in range(4)]  # p01,p23,p45,p6
        s_t7 = sem("s_t7")
        s_p7 = sem("s_p7")
        s_mset = sem("s_mset")
        s_h1 = sem("s_h1")
        s_psA = sem("s_psA")
        s_eq = sem("s_eq")
        s_isops = sem("s_isops")
        s_z = sem("s_z")
        s_zmm0 = sem("s_zmm0")
        s_zmm1 = sem("s_zmm1")
        s_actd = sem("s_actd")
        s_dved = sem("s_dved")
        s_out = sem("s_out")

        with nc.Block() as block:

            @block.sync
            def _(sync):
                sync.dma_start(thr_t[:], thr[:]).then_inc(s_aux, 16)
                sync.dma_start(tri_t[:], tri[:]).then_inc(s_aux, 16)
                for j in range(3):
                    c = slice(2 * j, 2 * j + 2)
                    sync.dma_start(pt[:, c, W : 2 * W], t_v[:, c, :]).then_inc(
                        s_td[j], 16
                    )
                    sync.dma_start(pt[:, c, 0:W], p_v[:, c, :]).then_inc(
                        s_pd[j], 16
                    )
                sync.dma_start(pt[:, 6, W : 2 * W], t_v[:, 6, :]).then_inc(
                    s_td[3], 16
                )
                sync.dma_start(pt[:, 6, 0:W], p_v[:, 6, :]).then_inc(s_pd[3], 16)
                sync.dma_start(pt[:, 7, W : 2 * W], t_v[:, 7, :]).then_inc(
                    s_t7, 16
                )
                sync.dma_start(pt[:, 7, 0:W], p_v[:, 7, :]).then_inc(s_p7, 16)
                sync.wait_ge(s_actd, 1)
                sync.wait_ge(s_dved, 1)
                sync.dma_start(out_d[:], out_sb[:]).then_inc(s_out, 16)

            @block.gpsimd
            def _(gpsimd):
                nc.gpsimd.memset(mp[:, :, 0 : W + 2 : W + 1], 0.0)
                nc.gpsimd.memset(out_sb[:, 10:14], 0.0).then_inc(s_mset, 1)

            @block.vector
            def _(vector):
                # exact mask for img0 chunks 0,1 (arrive in pair 0)
                vector.wait_ge(s_pd[0], 16)
                nc.vector.tensor_scalar(
                    mp[:, :, 1 : W + 1], pt[:, 0:2, 0:W], thr_t[:], None, A.is_gt
                )
                vector.wait_ge(s_mset, 1)
                nc.vector.tensor_add(
                    h1[:], mp[:, :, 0:W], mp[:, :, 2 : W + 2]
                ).then_inc(s_h1, 1)

                def mul_pair(j, wait=True):
                    c = slice(2 * j, 2 * j + 2)
                    if wait:
                        vector.wait_ge(s_pd[j], 16)
                    return nc.vector.tensor_mul(
                        z_all[:, j, :, :], pt[:, c, 0:W], pt[:, c, W : 2 * W]
                    ).then_inc(s_z, 1)

                mul_pair(0, wait=False)  # pair 0 confirmed by the mask's wait
                vector.wait_ge(s_psA, 1)
                nc.vector.tensor_scalar(
                    ind[:, 0, :], psA[0][:], -1.0, None, A.is_equal
                ).then_inc(s_eq, 1)
                vector.wait_ge(s_psA, 2)
                nc.vector.tensor_scalar(
                    ind[:, 1, :], psA[1][:], -1.0, None, A.is_equal
                ).then_inc(s_eq, 1)
                mul_pair(1)
                mul_pair(2)
                vector.wait_ge(s_isops, 1)
                nc.vector.tensor_reduce(
                    out_sb[0:1, 13:14], iso_ps[:], axis=X, op=A.add
                )
                vector.wait_ge(s_zmm0, 1)
                nc.vector.tensor_reduce(
                    out_sb[0:1, 10:11], zps0[:], axis=X, op=A.add
                )
                vector.wait_ge(s_pd[3], 16)
                nc.vector.tensor_mul(
                    z_all[:, 3, 0, :], pt[:, 6, 0:W], pt[:, 6, W : 2 * W]
                ).then_inc(s_z, 1)
                vector.wait_ge(s_p7, 16)
                nc.vector.tensor_mul(
                    z_all[:, 3, 1, :], pt[:, 7, 0:W], pt[:, 7, W : 2 * W]
                )
                nc.vector.tensor_reduce(
                    out_sb[:, 12:13], z_all[:, 3, 1, :], axis=X, op=A.add
                )
                vector.wait_ge(s_zmm1, 1)
                nc.vector.tensor_reduce(
                    out_sb[0:1, 11:12], zps1[:], axis=X, op=A.add
                ).then_inc(s_dved, 1)

            @block.scalar
            def _(scalar):
                # per pair-half squares: t then p, pipelined with arrivals
                for j in range(3):
                    c = slice(2 * j, 2 * j + 2)
                    scalar.wait_ge(s_td[j], 16)
                    nc.scalar.activation(
                        sq_scr[:],
                        pt[:, c, W : 2 * W],
                        AF.Square,
                        accum_out=out_sb[:, 2 * j : 2 * j + 1],
                    )
                    scalar.wait_ge(s_pd[j], 16)
                    nc.scalar.activation(
                        sq_scr[:],
                        pt[:, c, 0:W],
                        AF.Square,
                        accum_out=out_sb[:, 2 * j + 1 : 2 * j + 2],
                    )
                scalar.wait_ge(s_td[3], 16)
                nc.scalar.activation(
                    sq_scr[:, 0, :], pt[:, 6, W : 2 * W], AF.Square,
                    accum_out=out_sb[:, 6:7],
                )
                scalar.wait_ge(s_pd[3], 16)
                nc.scalar.activation(
                    sq_scr[:, 0, :], pt[:, 6, 0:W], AF.Square,
                    accum_out=out_sb[:, 7:8],
                )
                scalar.wait_ge(s_t7, 16)
                nc.scalar.activation(
                    sq_scr[:, 0, :], pt[:, 7, W : 2 * W], AF.Square,
                    accum_out=out_sb[:, 8:9],
                )
                scalar.wait_ge(s_p7, 16)
                nc.scalar.activation(
                    sq_scr[:, 0, :], pt[:, 7, 0:W], AF.Square,
                    accum_out=out_sb[:, 9:10],
                ).then_inc(s_actd, 1)

            @block.tensor
            def _(tensor):
                T3 = tri_t[:, 0:128]
                C = tri_t[:, 128:256]
                U = tri_t[:, 256:384]
                ones127 = tri_t[:, 384:385]
                ones = tri_t[:, 385:386]
                mm = nc.tensor.matmul
                # chunk0: rows 0..126 valid (top edge exact, row 127 dropped)
                tensor.wait_ge(s_h1, 1)
                mm(psA[0][:], T3, h1[:, 0, :], start=True, stop=False,
                   skip_group_check=True)
                mm(psA[0][:], C, mp[:, 0, 1 : W + 1], start=False, stop=True,
                   skip_group_check=True).then_inc(s_psA, 1)
                # chunk1: U fix pulls chunk0 row 127 into row 0's window
                mm(psA[1][:], T3, h1[:, 1, :], start=True, stop=False,
                   skip_group_check=True)
                mm(psA[1][:], C, mp[:, 1, 1 : W + 1], start=False, stop=False,
                   skip_group_check=True)
                mm(psA[1][:], U, h1[:, 0, :], start=False, stop=False,
                   skip_group_check=True)
                mm(psA[1][:], U, mp[:, 0, 1 : W + 1], start=False, stop=True,
                   skip_group_check=True).then_inc(s_psA, 1)
                # z count matmuls, pair 0 (img0)
                tensor.wait_ge(s_z, 1)
                mm(zps0[:], ones, z_all[:, 0, 0, :], start=True, stop=False,
                   skip_group_check=True)
                mm(zps0[:], ones, z_all[:, 0, 1, :], start=False, stop=False,
                   skip_group_check=True)
                # iso counts (rows 0..126 of both chunks)
                tensor.wait_ge(s_eq, 1)
                mm(iso_ps[:], ones127, ind[:, 0, :], start=True, stop=False,
                   skip_group_check=True)
                tensor.wait_ge(s_eq, 2)
                mm(iso_ps[:], ones127, ind[:, 1, :], start=False, stop=True,
                   skip_group_check=True).then_inc(s_isops, 1)
                # pair 1 completes img0
                tensor.wait_ge(s_z, 2)
                mm(zps0[:], ones, z_all[:, 1, 0, :], start=False, stop=False,
                   skip_group_check=True)
                mm(zps0[:], ones, z_all[:, 1, 1, :], start=False, stop=True,
                   skip_group_check=True).then_inc(s_zmm0, 1)
                # img1: pair 2 + chunk 6
                tensor.wait_ge(s_z, 3)
                mm(zps1[:], ones, z_all[:, 2, 0, :], start=True, stop=False,
                   skip_group_check=True)
                mm(zps1[:], ones, z_all[:, 2, 1, :], start=False, stop=False,
                   skip_group_check=True)
                tensor.wait_ge(s_z, 4)
                mm(zps1[:], ones, z_all[:, 3, 0, :], start=False, stop=True,
                   skip_group_check=True).then_inc(s_zmm1, 1)

        nc.compile()
    return nc


def _get_built():
    if "nc" not in _cache:
        _cache["nc"] = _build()
    return _cache["nc"]


def kernel(predict, target):
    import os

    from concourse.bass_utils import run_bass_kernel_spmd

    trace = bool(os.environ.get("BDICE_TRACE"))
    if trace:
        _install_ntff_hook()

    pred = np.ascontiguousarray(np.asarray(predict, np.float32).reshape(B * H, W))
    targ = np.ascontiguousarray(np.asarray(target, np.float32).reshape(B * H, W))
    p_sh = pred.reshape(N_CORES, RPC, W)
    t_sh = targ.reshape(N_CORES, RPC, W)

    thr_f32 = np.float32(pred.max()) / np.float32(2.0)
    thr_arr = np.full((128, 1), thr_f32, np.float32)

    nc = _get_built()
    core_ids = list(range(N_CORES))
    tri = _tri_matrices()
    in_maps = [
        {"p": p_sh[c], "t": t_sh[c], "tri": tri, "thr": thr_arr}
        for c in range(N_CORES)
    ]
    res = run_bass_kernel_spmd(nc, in_maps, core_ids=core_ids, trace=trace)
    if trace:
        LAST_PERF.update(
            a_ns=res.exec_time_ns,
            b_ns=0,
            a_trace=(res.instructions_and_trace or (None, None))[1],
            b_trace=None,
        )

    out = np.stack([res.results[c]["out"] for c in range(N_CORES)]).astype(
        np.float64
    )

    iso_total = float(out[:, 0, 13].sum())

    losses = []
    for c in range(N_CORES):
        den0 = out[c, :, 0:4].sum()
        den1 = out[c, :, 4:10].sum()
        num0 = out[c, 0, 10]
        num1 = out[c, 0, 11] + out[c, :, 12].sum()
        losses.append(1.0 - (num0 + 1.0) / (den0 + 1.0))
        losses.append(1.0 - (num1 + 1.0) / (den1 + 1.0))
    mean_loss = float(np.mean(losses))

    if iso_total >= 254.5:
        penalty = 16.0
    else:
        penalty = _penalty_fallback(pred)

    return np.float32(mean_loss * penalty)


# revision 13
# speedup vs baseline: 1.8956x; 1.0447x over previous
"""Trainium2 Bass kernel for nn_BinaryDiceLoss_blobPunish (B=16, H=W=512).

Reference semantics:
    thr = predict.max()/2;  mask = predict > thr
    labels = 200 iters of masked 3x3 max-pool label propagation
    n_unique = #distinct label values
    penalty = clip: n_unique/B, <1 -> B, capped at B
    dice_i = 1 - (sum(p_i t_i)+1)/(sum(p_i^2)+sum(t_i^2)+1)
    out = mean(dice_i) * penalty

Distribution: 2 images per core on 8 NeuronCores, ONE SPMD launch, no
cross-core collectives.

Host computes the EXACT threshold thr = max(predict)/2 in f32 (bit-
identical to the reference) and ships it as a tiny input, so the
device mask is exact.  Every isolated mask pixel (no 8-neighbors set)
keeps its own unique label under max-pool propagation, and background
0 is present whenever an isolated pixel exists, so
    n_unique >= iso_count + 1.
The device counts isolated pixels on rows 0..126 of the first two
128-row chunks of image 0 on each core (exact on those rows; row 127
is excluded via a zeroed lane in the ones column).  Expected count
~2100 >> 255; if the count ever drops below 255 an exact numpy
fallback recomputes the penalty (never hit for this generator).

Dice sums:  ACT Square+accum per t/p pair-half gives sum(t^2)/sum(p^2)
pipelined against the staggered t/p DMA arrivals.  DVE tensor_mul
(bf16 out) + PE ones-column matmuls accumulate sum(p*t) into PSUM;
the final chunk's z is reduced directly on DVE to keep the tail short.
Isolated-pixel test: m = mask (bf16, zero-padded borders),
H1 = m_left + m_right (DVE), then PE band matmuls build
S3x3 - 2m = T3 @ H1 + (T3 - 2I) @ m (+ U cross-chunk row fix);
a pixel is isolated iff that equals -1 (DVE is_equal), counted with a
ones(0..126) column matmul into PSUM and a final DVE reduce.

Raw-bacc implementation (no TileContext): hand-placed semaphores.
All input DMAs ride ONE SP hardware queue (FIFO completion): 2-chunk
pairs (t-half then p-half) for chunks 0-5, then single chunks 6,7 so
the tail work is fine-grained.  Kernel is HBM-bound (~4.2 MB/core).

Engine programs (all in-order per engine, <=1 sem wait per instruction):
  SP  : thr, tri, t01,p01,t23,p23,t45,p45,t6,p6,t7,p7 dmas, 1 out dma
  GP  : mask border + out_sb tail-column memsets only
  DVE : mask, H1, 4 z muls, 2 iso is_equal, iso/zps/z7 reduces
  ACT : 6 pair-half + 4 chunk Square accums
  PE  : 6 cert band matmuls + 2 iso count + 5 z count matmuls
"""

from contextlib import ExitStack

import numpy as np

B = 16
H = 512
W = 512
N_CORES = 8
IPC = B // N_CORES  # images per core
RPC = IPC * H  # rows per core
NCHUNK = RPC // 128  # 8 128-row chunks per core


def _install_ntff_hook():
    """Make trace=True work under axon: the stub antenv package lacks
    axon_hooks, so boot() silently skipped NTFF hook registration."""
    import sys
    import types

    if "antenv.axon_hooks" in sys.modules:
        return
    try:
        import antenv

        mod = types.ModuleType("antenv.axon_hooks")
        mod._hook = None
        mod.set_axon_ntff_profile_hook = lambda h: setattr(mod, "_hook", h)
        mod.get_axon_ntff_profile_hook = lambda: mod._hook
        sys.modules["antenv.axon_hooks"] = mod
        antenv.axon_hooks = mod
        from trn_agent_boot.trn_boot import _ntff_profile_via_ctypes

        hook = _ntff_profile_via_ctypes("/opt/axon/libaxon_pjrt.so")
        if hook is not None:
            mod.set_axon_ntff_profile_hook(hook)
    except Exception:
        pass


def _tri_matrices():
    import ml_dtypes

    tri = np.zeros((128, 3 * 128 + 2), np.float32)
    idx = np.arange(128)
    T3 = tri[:, 0:128]
    T3[idx, idx] = 1.0
    T3[idx[:-1], idx[:-1] + 1] = 1.0
    T3[idx[:-1] + 1, idx[:-1]] = 1.0
    C = tri[:, 128:256]
    C[:] = T3
    C[idx, idx] = -1.0
    tri[127, 256 + 0] = 1.0  # U: lhsT[127,0] -> out row 0 += rhs row 127
    tri[0:127, 384] = 1.0  # ones column, row 127 zeroed (excluded rows)
    tri[:, 385] = 1.0  # full ones column for z count matmuls
    return tri.astype(ml_dtypes.bfloat16)


def _penalty_fallback(predict):
    """Exact numpy replica of the reference penalty path (rarely used)."""
    p = np.asarray(predict, np.float32).reshape(B, H, W)
    thr = np.float32(p.max()) / np.float32(2.0)
    mask = p > thr
    init = np.arange(B * H * W, dtype=np.float32).reshape(B, H, W)
    lab = np.where(mask, init, np.float32(0.0))
    pad = np.empty((B, H + 2, W + 2), np.float32)
    for _ in range(200):
        pad.fill(-np.inf)
        pad[:, 1:-1, 1:-1] = lab
        mx = pad[:, 0:-2, 0:-2]
        for dr in range(3):
            for dc in range(3):
                if dr == 0 and dc == 0:
                    continue
                mx = np.maximum(mx, pad[:, dr : dr + H, dc : dc + W])
        new = np.where(mask, mx, np.float32(0.0))
        if np.array_equal(new, lab):
            lab = new
            break
        lab = new
    n_unique = np.unique(lab).size
    penalty = np.float32(n_unique) / np.float32(B)
    if penalty < 1.0:
        penalty = np.float32(B)
    return float(min(penalty, np.float32(B)))


_cache: dict = {}
LAST_PERF: dict = {}


def _build():
    import concourse.bacc as bacc
    from concourse import mybir

    f32 = mybir.dt.float32
    bf16 = mybir.dt.bfloat16
    A = mybir.AluOpType
    AF = mybir.ActivationFunctionType
    X = mybir.AxisListType.X

    nc = bacc.Bacc("TRN2", target_bir_lowering=False, debug=False, num_devices=N_CORES)
    p = nc.dram_tensor("p", [RPC, W], f32, kind="ExternalInput").ap()
    t = nc.dram_tensor("t", [RPC, W], f32, kind="ExternalInput").ap()
    tri = nc.dram_tensor("tri", [128, 3 * 128 + 2], bf16, kind="ExternalInput").ap()
    thr = nc.dram_tensor("thr", [128, 1], f32, kind="ExternalInput").ap()
    out_d = nc.dram_tensor("out", [128, 14], f32, kind="ExternalOutput").ap()

    # partition-major views: [q=partition, n=chunk, m=col]
    p_v = p.rearrange("(n q) m -> q n m", q=128)
    t_v = t.rearrange("(n q) m -> q n m", q=128)

    with ExitStack() as ctx:
        _n = [0]

        def sb(shape, dt, name=None):
            _n[0] += 1
            return ctx.enter_context(
                nc.sbuf_tensor(name or f"sb{_n[0]}", shape, dt)
            )

        def ps(shape, name=None):
            _n[0] += 1
            return ctx.enter_context(
                nc.psum_tensor(name or f"ps{_n[0]}", shape, f32)
            )

        def sem(name):
            return ctx.enter_context(nc.semaphore(name))

        tri_t = sb([128, 3 * 128 + 2], bf16)
        thr_t = sb([128, 1], f32)
        # interleaved blocks: chunk j of p at [:, j, 0:W], t at [:, j, W:2W]
        pt = sb([128, NCHUNK, 2 * W], f32)
        mp = sb([128, 2, W + 2], bf16)  # img0 chunks 0,1 mask + borders
        h1 = sb([128, 2, W], bf16)
        ind = sb([128, 2, W], bf16)
        z_all = sb([128, 4, 2, W], bf16)  # elementwise p*t per pair
        sq_scr = sb([128, 2, W], bf16)  # ACT square scratch
        out_sb = sb([128, 14], f32)

        psA = [ps([128, W]) for _ in range(2)]
        iso_ps = ps([1, W])
        zps0 = ps([1, W])  # img0: z pairs 0,1
        zps1 = ps([1, W])  # img1: z pair 2 + chunk 6

        s_aux = sem("s_aux")
        s_td = [sem(f"s_td{j}") for j in range(4)]  # t01,t23,t45,t6
        s_pd = [sem(f"s_pd{j}") for j in range(4)]  # p01,p23,p45,p6
        s_t7 = sem("s_t7")
        s_p7 = sem("s_p7")
        s_mset = sem("s_mset")
        s_h1 = sem("s_h1")
        s_psA = sem("s_psA")
        s_eq = sem("s_eq")
        s_isops = sem("s_isops")
        s_z = sem("s_z")
        s_zmm0 = sem("s_zmm0")
        s_zmm1 = sem("s_zmm1")
        s_actd = sem("s_actd")
        s_dved = sem("s_dved")
        s_out = sem("s_out")

        with nc.Block() as block:

            @block.sync
            def _(sync):
                # pair 0 first so squares/muls start ASAP; thr/tri ride
                # after it (mask waits s_aux>=32, FIFO covers t01/p01 too)
                for j in range(3):
                    c = slice(2 * j, 2 * j + 2)
                    sync.dma_start(pt[:, c, W : 2 * W], t_v[:, c, :]).then_inc(
                        s_td[j], 16
                    )
                    sync.dma_start(pt[:, c, 0:W], p_v[:, c, :]).then_inc(
                        s_pd[j], 16
                    )
                    if j == 0:
                        sync.dma_start(thr_t[:], thr[:]).then_inc(s_aux, 16)
                        sync.dma_start(tri_t[:], tri[:]).then_inc(s_aux, 16)
                sync.dma_start(pt[:, 6, W : 2 * W], t_v[:, 6, :]).then_inc(
                    s_td[3], 16
                )
                sync.dma_start(pt[:, 6, 0:W], p_v[:, 6, :]).then_inc(s_pd[3], 16)
                sync.dma_start(pt[:, 7, W : 2 * W], t_v[:, 7, :]).then_inc(
                    s_t7, 16
                )
                sync.dma_start(pt[:, 7, 0:W], p_v[:, 7, :]).then_inc(s_p7, 16)
                sync.wait_ge(s_actd, 1)
                sync.wait_ge(s_dved, 1)
                sync.dma_start(out_d[:], out_sb[:]).then_inc(s_out, 16)

            @block.gpsimd
            def _(gpsimd):
                nc.gpsimd.memset(mp[:, :, 0 : W + 2 : W + 1], 0.0)
                nc.gpsimd.memset(out_sb[:, 10:14], 0.0).then_inc(s_mset, 1)

            @block.vector
            def _(vector):
                # exact mask for img0 chunks 0,1 (arrive in pair 0)
                vector.wait_ge(s_aux, 32)
                nc.vector.tensor_scalar(
                    mp[:, :, 1 : W + 1], pt[:, 0:2, 0:W], thr_t[:], None, A.is_gt
                )
                vector.wait_ge(s_mset, 1)
                nc.vector.tensor_add(
                    h1[:], mp[:, :, 0:W], mp[:, :, 2 : W + 2]
                ).then_inc(s_h1, 1)

                def mul_pair(j, wait=True):
                    c = slice(2 * j, 2 * j + 2)
                    if wait:
                        vector.wait_ge(s_pd[j], 16)
                    return nc.vector.tensor_mul(
                        z_all[:, j, :, :], pt[:, c, 0:W], pt[:, c, W : 2 * W]
                    ).then_inc(s_z, 1)

                mul_pair(0, wait=False)  # pair 0 confirmed by the mask's wait
                vector.wait_ge(s_psA, 1)
                nc.vector.tensor_scalar(
                    ind[:, 0, :], psA[0][:], -1.0, None, A.is_equal
                ).then_inc(s_eq, 1)
                vector.wait_ge(s_psA, 2)
                nc.vector.tensor_scalar(
                    ind[:, 1, :], psA[1][:], -1.0, None, A.is_equal
                ).then_inc(s_eq, 1)
                mul_pair(1)
                mul_pair(2)
                vector.wait_ge(s_isops, 1)
                nc.vector.tensor_reduce(
                    out_sb[0:1, 13:14], iso_ps[:], axis=X, op=A.add
                )
                vector.wait_ge(s_zmm0, 1)
                nc.vector.tensor_reduce(
                    out_sb[0:1, 10:11], zps0[:], axis=X, op=A.add
                )
                vector.wait_ge(s_pd[3], 16)
                nc.vector.tensor_mul(
                    z_all[:, 3, 0, :], pt[:, 6, 0:W], pt[:, 6, W : 2 * W]
                ).then_inc(s_z, 1)
                vector.wait_ge(s_p7, 16)
                nc.vector.tensor_mul(
                    z_all[:, 3, 1, :], pt[:, 7, 0:W], pt[:, 7, W : 2 * W]
                )
                nc.vector.tensor_reduce(
                    out_sb[:, 12:13], z_all[:, 3, 1, :], axis=X, op=A.add
                )
                vector.wait_ge(s_zmm1, 1)
                nc.vector.tensor_reduce(
                    out_sb[0:1, 11:12], zps1[:], axis=X, op=A.add
                ).then_inc(s_dved, 1)

            @block.scalar
            def _(scalar):
                # per pair-half squares: t then p, pipelined with arrivals
                for j in range(3):
                    c = slice(2 * j, 2 * j + 2)
                    scalar.wait_ge(s_td[j], 16)
                    nc.scalar.activation(
                        sq_scr[:],
                        pt[:, c, W : 2 * W],
                        AF.Square,
                        accum_out=out_sb[:, 2 * j : 2 * j + 1],
                    )
                    scalar.wait_ge(s_pd[j], 16)
                    nc.scalar.activation(
                        sq_scr[:],
                        pt[:, c, 0:W],
                        AF.Square,
                        accum_out=out_sb[:, 2 * j + 1 : 2 * j + 2],
                    )
                scalar.wait_ge(s_td[3], 16)
                nc.scalar.activation(
                    sq_scr[:, 0, :], pt[:, 6, W : 2 * W], AF.Square,
                    accum_out=out_sb[:, 6:7],
                )
                scalar.wait_ge(s_pd[3], 16)
                nc.scalar.activation(
                    sq_scr[:, 0, :], pt[:, 6, 0:W], AF.Square,
                    accum_out=out_sb[:, 7:8],
                )
                scalar.wait_ge(s_t7, 16)
                nc.scalar.activation(
                    sq_scr[:, 0, :], pt[:, 7, W : 2 * W], AF.Square,
                    accum_out=out_sb[:, 8:9],
                )
                scalar.wait_ge(s_p7, 16)
                nc.scalar.activation(
                    sq_scr[:, 0, :], pt[:, 7, 0:W], AF.Square,
                    accum_out=out_sb[:, 9:10],
                ).then_inc(s_actd, 1)

            @block.tensor
            def _(tensor):
                T3 = tri_t[:, 0:128]
                C = tri_t[:, 128:256]
                U = tri_t[:, 256:384]
                ones127 = tri_t[:, 384:385]
                ones = tri_t[:, 385:386]
                mm = nc.tensor.matmul
                # chunk0: rows 0..126 valid (top edge exact, row 127 dropped)
                tensor.wait_ge(s_h1, 1)
                mm(psA[0][:], T3, h1[:, 0, :], start=True, stop=False,
                   skip_group_check=True)
                mm(psA[0][:], C, mp[:, 0, 1 : W + 1], start=False, stop=True,
                   skip_group_check=True).then_inc(s_psA, 1)
                # chunk1: U fix pulls chunk0 row 127 into row 0's window
                mm(psA[1][:], T3, h1[:, 1, :], start=True, stop=False,
                   skip_group_check=True)
                mm(psA[1][:], C, mp[:, 1, 1 : W + 1], start=False, stop=False,
                   skip_group_check=True)
                mm(psA[1][:], U, h1[:, 0, :], start=False, stop=False,
                   skip_group_check=True)
                mm(psA[1][:], U, mp[:, 0, 1 : W + 1], start=False, stop=True,
                   skip_group_check=True).then_inc(s_psA, 1)
                # z count matmuls, pair 0 (img0)
                tensor.wait_ge(s_z, 1)
                mm(zps0[:], ones, z_all[:, 0, 0, :], start=True, stop=False,
                   skip_group_check=True)
                mm(zps0[:], ones, z_all[:, 0, 1, :], start=False, stop=False,
                   skip_group_check=True)
                # iso counts (rows 0..126 of both chunks)
                tensor.wait_ge(s_eq, 1)
                mm(iso_ps[:], ones127, ind[:, 0, :], start=True, stop=False,
                   skip_group_check=True)
                tensor.wait_ge(s_eq, 2)
                mm(iso_ps[:], ones127, ind[:, 1, :], start=False, stop=True,
                   skip_group_check=True).then_inc(s_isops, 1)
                # pair 1 completes img0
                tensor.wait_ge(s_z, 2)
                mm(zps0[:], ones, z_all[:, 1, 0, :], start=False, stop=False,
                   skip_group_check=True)
                mm(zps0[:], ones, z_all[:, 1, 1, :], start=False, stop=True,
                   skip_group_check=True).then_inc(s_zmm0, 1)
                # img1: pair 2 + chunk 6
                tensor.wait_ge(s_z, 3)
                mm(zps1[:], ones, z_all[:, 2, 0, :], start=True, stop=False,
                   skip_group_check=True)
                mm(zps1[:], ones, z_all[:, 2, 1, :], start=False, stop=False,
                   skip_group_check=True)
                tensor.wait_ge(s_z, 4)
                mm(zps1[:], ones, z_all[:, 3, 0, :], start=False, stop=True,
                   skip_group_check=True).then_inc(s_zmm1, 1)

        nc.compile()
    return nc


def _get_built():
    if "nc" not in _cache:
        _cache["nc"] = _build()
    return _cache["nc"]


def kernel(predict, target):
    import os

    from concourse.bass_utils import run_bass_kernel_spmd

    trace = bool(os.environ.get("BDICE_TRACE"))
    if trace:
        _install_ntff_hook()

    pred = np.ascontiguousarray(np.asarray(predict, np.float32).reshape(B * H, W))
    targ = np.ascontiguousarray(np.asarray(target, np.float32).reshape(B * H, W))
    p_sh = pred.reshape(N_CORES, RPC, W)
    t_sh = targ.reshape(N_CORES, RPC, W)

    thr_f32 = np.float32(pred.max()) / np.float32(2.0)
    thr_arr = np.full((128, 1), thr_f32, np.float32)

    nc = _get_built()
    core_ids = list(range(N_CORES))
    tri = _tri_matrices()
    in_maps = [
        {"p": p_sh[c], "t": t_sh[c], "tri": tri, "thr": thr_arr}
        for c in range(N_CORES)
    ]
    res = run_bass_kernel_spmd(nc, in_maps, core_ids=core_ids, trace=trace)
    if trace:
        LAST_PERF.update(
            a_ns=res.exec_time_ns,
            b_ns=0,
            a_trace=(res.instructions_and_trace or (None, None))[1],
            b_trace=None,
        )

    out = np.stack([res.results[c]["out"] for c in range(N_CORES)]).astype(
        np.float64
    )

    iso_total = float(out[:, 0, 13].sum())

    losses = []
    for c in range(N_CORES):
        den0 = out[c, :, 0:4].sum()
        den1 = out[c, :, 4:10].sum()
        num0 = out[c, 0, 10]
        num1 = out[c, 0, 11] + out[c, :, 12].sum()
        losses.append(1.0 - (num0 + 1.0) / (den0 + 1.0))
        losses.append(1.0 - (num1 + 1.0) / (den1 + 1.0))
    mean_loss = float(np.mean(losses))

    if iso_total >= 254.5:
        penalty = 16.0
    else:
        penalty = _penalty_fallback(pred)

    return np.float32(mean_loss * penalty)
